# revision 28
# baseline (speedup 1.0000x reference)
"""Trainium2 Bass kernel for nn_AlignmentModule (conv stems + L2 score +
log-softmax + beta-binomial prior).

Sharding: 8 cores = 4 batches x 2 T_feats halves. Each core computes the
text conv stem for its batch (duplicated within a pair), its half of the
feats conv stem (halo rows come zero-padded from DRAM; the one halo column
of f1 that the reference's conv padding zeroes is masked on device), the
(400, 160) score block with log-softmax over T_text, adds the
(input-independent, host-precomputed) beta-binomial prior slice, and
writes its output block.

v2 (vs the fp32r baseline):
 - all matmuls run in fp16 (tolerance is 2e-2 fro; fp16 stems land ~1e-3).
   Halves input DMA bytes and drops the fp32r N>=256 rate constraint, so
   T_text matmuls stay at 160 columns unpadded.
 - the ACT engine uses a single activation table (sqrt_and_others:
   relu/identity/square/sqrt) for the whole kernel, warmed on memset
   scratch at t~0. exp/ln of the log-softmax run as Schraudolph bit-trick
   approximations on the DVE (z only enters the output through ln z, so
   its ~2.5% error contributes ~0.02 abs — far inside tolerance).
 - no max-subtraction: score = -sqrt(d2) <= 0 so exp(score) <= 1 never
   overflows, and s <= ~25 keeps it well above underflow.
 - d2 accumulates in two (100, 320) PSUM blocks (2 T_feats tiles each);
   |f|^2 and |t|^2+mask ride one fused K=2 augmentation matmul.
 - epilogue is spread across ACT (sqrt), DVE (exp-bits, z reduce, ln),
   and Pool (prior-s, -lnz) in 320-column ops.

Self-contained: hardcodes all shapes; reads nothing from disk.
"""

import math
import os
import subprocess
import sys

import numpy as np

import concourse.bass as bass
import concourse.mybir as mybir
import concourse.tile as tile
from concourse.bass_utils import run_bass_kernel_spmd

B, T_TEXT, T_FEATS = 4, 160, 800
ADIM, ODIM = 256, 80
N_CORES = 8
HALF = T_FEATS // 2          # 400 feats rows per core
TT = T_TEXT                  # 160
TTW = TT + 2                 # 162: text conv window
TFW = HALF + 2               # 402: f1 window  [s-1, s+401)
TFIN = HALF + 4              # 404: feats input window [s-2, s+402)
MT = 100                     # T_feats tile rows per score tile
NMT = HALF // MT             # 4 score tiles
NB = 2                       # d2 psum blocks (2 tiles each)
F32 = mybir.dt.float32
F16 = mybir.dt.float16
I32 = mybir.dt.int32
MASK_PENALTY = 4000.0        # d2 offset for masked cols (fp16/exp-bit safe)

# Schraudolph exp/log constants (validated on-device):
#   exp(-s) ~= bits_as_f32(round(-A_EXP*s + B_EXP))
#   ln(z)   ~= LN_MUL * float(bits_of(z)) + LN_ADD
LOG2E = 1.4426950408889634
A_EXP = float(2 ** 23) * LOG2E
B_EXP = float(2 ** 23) * 127 - 366393.0
LN_MUL = math.log(2.0) / float(2 ** 23)
LN_ADD = -math.log(2.0) * (127.0 - 366393.0 / float(2 ** 23))

# biasmask pack layout (128 partitions, f32)
BM_BIAS = 0                  # cols 0..15: bias j of ci-chunk c at 8c+j
BM_MASK2 = 16                # cols 16..17: f1 halo-column masks
BM_TMROW = 18                # cols 18..18+TT: x_mask penalty row (row 0)
BM_W = BM_TMROW + TT
TB1, TB2, FB1, FB2, FB3, FB3M2 = range(6)

# fp16 pack layouts
FP_FEATS = 0                 # fpack: featsT at 0..404, fw1 after
FP_FW1 = TFIN
FP_W = TFIN + 3 * ADIM

TP_TEXT = 0                  # tpack: textT (2 chunks x TTW), tw1 after
TP_TW1 = 2 * TTW
TP_W = 2 * TTW + 2 * 768

F2P_W = 2 * 768              # f2pack: fw2 only

WD_TW2 = 0                   # wdpack: tw2 | fw3
WD_FW3 = 2 * 256
WD_W = 4 * 256

_nc_cache = None
_prior_cache = None


# ---------------------------------------------------------------- host math
def _prior_f64():
    """f64 fallback replica of reference.beta_binomial_prior."""
    try:
        from scipy.special import gammaln as _gl
    except Exception:
        _gl = np.vectorize(math.lgamma)
    T, N = float(T_FEATS), float(T_TEXT)
    a = np.arange(1, T_FEATS + 1, dtype=np.float64)
    b = T - a + 1.0
    k = np.arange(T_TEXT, dtype=np.float64)[:, None]

    def betaln(x, y):
        return _gl(x) + _gl(y) - _gl(x + y)

    logp = (
        _gl(N + 1.0) - _gl(k + 1.0) - _gl(N - k + 1.0)
        + betaln(k + a, N - k + b) - betaln(a, b)
    )
    return np.asarray(logp.T, dtype=np.float32)


_PRIOR_SRC = """
import os
os.environ["JAX_PLATFORMS"] = "cpu"
import numpy as np
import jax.numpy as jnp
from jax.scipy.special import gammaln

T, N = {T}, {N}
a = 1.0 * jnp.arange(1, T + 1, dtype=jnp.float32)
b = 1.0 * (T - a + 1.0)
k = jnp.arange(N, dtype=jnp.float32)[:, None]
Nf = jnp.float32(N)

def betaln(x, y):
    return gammaln(x) + gammaln(y) - gammaln(x + y)

logp = (gammaln(Nf + 1.0) - gammaln(k + 1.0) - gammaln(Nf - k + 1.0)
        + betaln(k + a, Nf - k + b) - betaln(a, b))
np.save({out!r}, np.asarray(logp.T, dtype=np.float32))
"""


def _beta_binomial_prior():
    """beta_binomial_prior(T_FEATS, T_TEXT), matching the reference's jax
    f32 computation. Computed once in a JAX_PLATFORMS=cpu subprocess,
    cached on disk and in-process."""
    global _prior_cache
    if _prior_cache is not None:
        return _prior_cache
    cache = f"/tmp/_bbprior_{T_FEATS}x{T_TEXT}.npy"
    if not os.path.exists(cache):
        src = _PRIOR_SRC.format(T=T_FEATS, N=T_TEXT, out=cache)
        for attempt in range(4):
            try:
                env = dict(os.environ)
                env.pop("JAX_PLATFORMS", None)
                r = subprocess.run([sys.executable, "-c", src],
                                   capture_output=True, timeout=600, env=env)
                if r.returncode == 0 and os.path.exists(cache):
                    break
                with open("/tmp/_bbprior_err.log", "ab") as f:
                    f.write(b"rc=%d\n" % r.returncode + r.stderr[-4000:])
            except Exception as e:
                try:
                    with open("/tmp/_bbprior_err.log", "a") as f:
                        f.write(f"attempt {attempt}: {e}\n")
                except Exception:
                    pass
    if os.path.exists(cache):
        _prior_cache = np.load(cache).astype(np.float32)
    else:
        _prior_cache = _prior_f64()
    return _prior_cache


# ------------------------------------------------------------- BIR patching
def _split_multiwait(nc):
    """This container's walrus accepts at most one sync wait per
    instruction; move extras onto single-wait NOPs just before."""
    for f in nc.m.functions:
        for bb in f.blocks:
            changed = False
            out = []
            for inst in bb.instructions:
                si = inst.sync_info
                if si is not None and len(si.on_wait) > 1:
                    waits = list(si.on_wait)
                    for j, w in enumerate(waits[:-1]):
                        nop = mybir.InstNoOp(name=f"{inst.name}sw{j}")
                        nop.name = f"{inst.name}sw{j}"
                        nop.engine = inst.engine
                        nop.sync_info = mybir.SyncInfo(on_wait=[w], on_update=[])
                        out.append(nop)
                    inst.sync_info = mybir.SyncInfo(
                        on_wait=[waits[-1]], on_update=list(si.on_update)
                    )
                    changed = True
                out.append(inst)
            if changed:
                bb.instructions = out


# ------------------------------------------------------------ device program
def _build_program():
    global _nc_cache
    if _nc_cache is not None:
        return _nc_cache

    nc = bass.Bass("TRN2", target_bir_lowering=False, debug=False,
                   num_devices=N_CORES, enable_asserts=False)
    AF = mybir.ActivationFunctionType
    AX = mybir.AxisListType
    AL = mybir.AluOpType

    d_bm = nc.dram_tensor("biasmask", [128, BM_W], F32, kind="ExternalInput")
    d_fp = nc.dram_tensor("fpack", [ODIM, FP_W], F16, kind="ExternalInput")
    d_f2 = nc.dram_tensor("f2pack", [128, F2P_W], F16, kind="ExternalInput")
    d_tp = nc.dram_tensor("tpack", [128, TP_W], F16, kind="ExternalInput")
    d_wd = nc.dram_tensor("wdpack", [128, WD_W], F16, kind="ExternalInput")
    d_pr = nc.dram_tensor("prior", [MT, NMT * TT], F32, kind="ExternalInput")
    d_out = nc.dram_tensor("out", [MT, NMT * TT], F32, kind="ExternalOutput")

    with tile.TileContext(nc) as tc:
        with (
            tc.tile_pool(name="dpool", bufs=1) as dpool,
            tc.tile_pool(name="spool", bufs=1) as spool,
            tc.tile_pool(name="epool", bufs=1) as epool,
            tc.tile_pool(name="psum", bufs=4, space="PSUM") as psum,
            tc.tile_pool(name="psumn", bufs=1, space="PSUM") as psumn,
            tc.tile_pool(name="psumd", bufs=2, space="PSUM") as psumd,
            tc.tile_pool(name="psumw", bufs=1, space="PSUM") as psumw,
        ):
            # -------- warm scratch + ACT table warm (no DMA deps) --------
            # scr is memset on gpsimd (fast, runs right after the start
            # barrier) so the ACT table load isn't gated on any DMA.
            scr = spool.tile([1, 4], F32, name="scr")
            nc.gpsimd.memset(scr[:], 1.0)
            wscr = spool.tile([128, 512], F16, name="wscr")
            nc.vector.memset(wscr[:], 1.0)
            # Sqrt first so the PWP resident table is sqrt_and_others;
            # every later ACT func (relu/identity/square) is in it too.
            nc.scalar.activation(scr[0:1, 1:2], scr[0:1, 0:1], AF.Sqrt)
            nc.scalar.activation(scr[0:1, 2:3], scr[0:1, 0:1], AF.Relu)

            # ones / constant rows (device memsets, no DMA)
            ones_col = spool.tile([128, 1], F16, name="ones_col")
            nc.gpsimd.memset(ones_col[:], 1.0)
            quart_col = spool.tile([128, 1], F16, name="quart_col")
            nc.gpsimd.memset(quart_col[:], 0.25)
            bexp_col = spool.tile([MT, 1], F32, name="bexp_col")
            nc.gpsimd.memset(bexp_col[:], B_EXP)
            ones_row = spool.tile([1, MT], F16, name="ones_row")
            nc.gpsimd.memset(ones_row[:], 1.0)

            # ---------------- input DMAs --------------------------------
            # DMA rings have ~2.2us startup latency; after that the two
            # hardware DGE queues (sync/scalar) stream at ~200GB/s each.
            # The gpsimd software DGE is slow and its teardown drain waits
            # on its whole queue, so it only carries the late-needed prior.
            fp = dpool.tile([ODIM, FP_W], F16, name="fp")
            nc.sync.dma_start(fp[:], d_fp.ap())
            tp = dpool.tile([128, TP_W], F16, name="tp")
            nc.sync.dma_start(tp[:], d_tp.ap())
            wd = dpool.tile([128, WD_W], F16, name="wd")
            nc.scalar.dma_start(wd[:], d_wd.ap())
            bm = dpool.tile([128, BM_W], F32, name="bm")
            nc.scalar.dma_start(bm[:], d_bm.ap())
            f2w = dpool.tile([128, F2P_W], F16, name="f2w")
            nc.scalar.dma_start(f2w[:], d_f2.ap())
            prior_sb = dpool.tile([MT, NMT * TT], F32, name="prior_sb")
            nc.gpsimd.dma_start(prior_sb[:], d_pr.ap())

            def bias(c, j):
                return bm[:, 8 * c + j: 8 * c + j + 1]

            # PE p-state ramp warm-up on memset scratch while fpack lands
            pwarm = psumw.tile([128, 384], F32, name="pwarm")

            def warm_mms(n, cols=384):
                for _ in range(n):
                    nc.tensor.matmul(pwarm[:, 0:cols], wscr[:, 0:128],
                                     wscr[:, 0:cols],
                                     start=True, stop=True,
                                     skip_group_check=True)

            warm_mms(4)

            # ---------------- feats conv1 -------------------------------
            f1_sb = []
            for co in range(2):
                p = psum.tile([128, TFW], F32, name=f"pf1_{co}", tag="convp")
                for k in range(3):
                    nc.tensor.matmul(
                        p[:],
                        fp[:, FP_FW1 + 256 * k + 128 * co:
                           FP_FW1 + 256 * k + 128 * (co + 1)],
                        fp[:, k:k + TFW],
                        start=(k == 0), stop=(k == 2),
                    )
                f1 = spool.tile([128, TFW], F16, name=f"f1_{co}")
                if co == 0:
                    nc.scalar.activation(f1[:], p[:], AF.Relu, bias=bias(co, FB1))
                else:
                    nc.vector.tensor_scalar(f1[:], p[:], bias(co, FB1), 0.0,
                                            op0=AL.add, op1=AL.max)
                # zero the halo column the reference conv padding zeroes
                # (on Pool as tensor_tensor: keeps the f2-gating cols off
                # the busy DVE queue; Pool AP-scalar ops are buggy but
                # tensor-tensor is fine)
                nc.gpsimd.tensor_mul(f1[:, 0:1], f1[:, 0:1],
                                     bm[:, BM_MASK2:BM_MASK2 + 1])
                nc.gpsimd.tensor_mul(f1[:, TFW - 1:TFW], f1[:, TFW - 1:TFW],
                                     bm[:, BM_MASK2 + 1:BM_MASK2 + 2])
                f1_sb.append(f1)

            # ---------------- text conv1 (fills f1-evac PE gap) ---------
            t1_sb = []
            for co in range(2):
                p = psum.tile([128, TT], F32, name=f"pt1_{co}", tag="convp",
                              padded_shape=[128, TFW])
                n = 0
                for ci in range(2):
                    for k in range(3):
                        nc.tensor.matmul(
                            p[:],
                            tp[:, TP_TW1 + 768 * ci + 256 * k + 128 * co:
                               TP_TW1 + 768 * ci + 256 * k + 128 * (co + 1)],
                            tp[:, TTW * ci + k: TTW * ci + k + TT],
                            start=(n == 0), stop=(n == 5),
                        )
                        n += 1
                t1 = spool.tile([128, TT], F16, name=f"t1_{co}")
                if co == 0:
                    nc.scalar.activation(t1[:], p[:], AF.Relu, bias=bias(co, TB1))
                else:
                    nc.vector.tensor_scalar(t1[:], p[:], bias(co, TB1), 0.0,
                                            op0=AL.add, op1=AL.max)
                t1_sb.append(t1)

            # ---------------- feats conv2 (before t2: only gated by the
            # f1 evacs and f2w; t2/tnorm then fill the f2-evac PE gap) ----
            f2_sb = []
            for co in range(2):
                p = psum.tile([128, HALF], F32, name=f"pf2_{co}", tag="convp",
                              padded_shape=[128, TFW])
                n = 0
                for ci in range(2):
                    for k in range(3):
                        nc.tensor.matmul(
                            p[:],
                            f2w[:, 768 * ci + 256 * k + 128 * co:
                                768 * ci + 256 * k + 128 * (co + 1)],
                            f1_sb[ci][:, k:k + HALF],
                            start=(n == 0), stop=(n == 5),
                        )
                        n += 1
                f2 = spool.tile([128, HALF], F16, name=f"f2_{co}")
                if co == 0:
                    nc.scalar.activation(f2[:], p[:], AF.Relu, bias=bias(co, FB2))
                else:
                    nc.vector.tensor_scalar(f2[:], p[:], bias(co, FB2), 0.0,
                                            op0=AL.add, op1=AL.max)
                f2_sb.append(f2)

            # ---------------- text conv2 + |t|^2 ------------------------
            t_sb, tt_sb = [], []
            for co in range(2):
                p = psum.tile([128, TT], F32, name=f"pt2_{co}", tag="convp",
                              padded_shape=[128, TFW])
                for ci in range(2):
                    nc.tensor.matmul(
                        p[:],
                        wd[:, WD_TW2 + 256 * ci + 128 * co:
                           WD_TW2 + 256 * ci + 128 * (co + 1)],
                        t1_sb[ci][:],
                        start=(ci == 0), stop=(ci == 1),
                    )
                t = spool.tile([128, TT], F16, name=f"t_{co}")
                tt = spool.tile([128, TT], F16, name=f"tt_{co}")
                if co == 0:
                    nc.scalar.activation(t[:], p[:], AF.Identity, bias=bias(co, TB2))
                    nc.scalar.activation(tt[:], p[:], AF.Square, bias=bias(co, TB2))
                else:
                    nc.vector.tensor_scalar_add(t[:], p[:], bias(co, TB2))
                    nc.vector.tensor_mul(tt[:], t[:], t[:])
                t_sb.append(t)
                tt_sb.append(tt)

            # |t|^2 + mask penalty -> aug_t row 1 (staged: tensor ops can't
            # write at partition offset 1, SBUF->SBUF DMA can; lands in the
            # f3/d2 matmul shadow)
            ptn = psumn.tile([1, TT], F32, name="ptn", tag="normp",
                             padded_shape=[MT, HALF])
            for ci in range(2):
                nc.tensor.matmul(ptn[:], ones_col[:], tt_sb[ci][:],
                                 start=(ci == 0), stop=(ci == 1))
            tn2m_row = spool.tile([1, TT], F16, name="tn2m_row")
            nc.vector.tensor_add(tn2m_row[:], ptn[:],
                                 bm[0:1, BM_TMROW:BM_TMROW + TT])

            # ---------------- feats conv3 -------------------------------
            # |f|^2 comes from fneg2^2 summed with a 0.25-scaled ones
            # column, so no separate (p+b)^2 evacuation is needed.
            fneg2_sb, ffq_sb = [], []
            for co in range(2):
                p = psum.tile([128, HALF], F32, name=f"pf3_{co}", tag="convp",
                              padded_shape=[128, TFW])
                for ci in range(2):
                    nc.tensor.matmul(
                        p[:],
                        wd[:, WD_FW3 + 256 * ci + 128 * co:
                           WD_FW3 + 256 * ci + 128 * (co + 1)],
                        f2_sb[ci][:],
                        start=(ci == 0), stop=(ci == 1),
                    )
                fneg2 = spool.tile([128, HALF], F16, name=f"fneg2_{co}")
                ffq = spool.tile([128, HALF], F16, name=f"ffq_{co}")
                if co == 0:
                    nc.scalar.activation(fneg2[:], p[:], AF.Identity, scale=-2.0,
                                         bias=bias(co, FB3M2))
                    nc.scalar.activation(ffq[:], fneg2[:], AF.Square)
                else:
                    nc.vector.tensor_scalar(fneg2[:], p[:], bias(co, FB3), -2.0,
                                            op0=AL.add, op1=AL.mult)
                    nc.vector.tensor_mul(ffq[:], fneg2[:], fneg2[:])
                fneg2_sb.append(fneg2)
                ffq_sb.append(ffq)

            # ---------------- d2 blocks + epilogue ----------------------
            s_all = epool.tile([MT, NMT * TT], F32, name="s_all")
            eb = epool.tile([MT, NMT * TT], I32, name="eb")
            o_all = epool.tile([MT, NMT * TT], F32, name="o_all")
            z4 = epool.tile([MT, NMT], F32, name="z4")
            zf4 = epool.tile([MT, NMT], F32, name="zf4")
            lnzn4 = epool.tile([MT, NMT], F32, name="lnzn4")

            BW = [3, 1]                       # tiles per d2 block
            BO = [0, 3]                       # first tile of each block
            pd2s = []
            for bk in range(NB):
                pd2 = psumd.tile([MT, BW[bk] * TT], F32, name=f"pd2_{bk}",
                                 tag="d2p", padded_shape=[MT, 3 * TT])
                pd2s.append(pd2)
            # |f|^2 transposed to [100, 4] columns: per tile, 2 K=128 matmuls
            # of ffq against the 0.25-ones column (it then rides the sqrt as
            # a per-partition bias, so d2 needs no |f|^2 rank-1 term at all)
            pfnt = psumn.tile([MT, NMT], F32, name="pfnt", tag="normp",
                              padded_shape=[MT, HALF])
            fn2t = spool.tile([MT, NMT], F32, name="fn2t")
            # d2' = |t|^2+mask (rank-1 from the partition-0 row, ready early)
            # - 2 f.t ; window-major brackets, fn2t columns interleaved
            for gi in range(NMT):
                bk = 0 if gi < 3 else 1
                pd2 = pd2s[bk]
                w = slice((gi - BO[bk]) * TT, (gi - BO[bk] + 1) * TT)
                lo = gi * MT
                nc.tensor.matmul(pd2[:, w], fneg2_sb[0][:, lo:lo + MT],
                                 t_sb[0][:], start=True, stop=False)
                nc.tensor.matmul(pd2[:, w], fneg2_sb[1][:, lo:lo + MT],
                                 t_sb[1][:], start=False, stop=False)
                nc.tensor.matmul(pd2[:, w], ones_row[:], tn2m_row[:],
                                 start=False, stop=True)
                for ci in range(2):
                    nc.tensor.matmul(pfnt[:, gi:gi + 1],
                                     ffq_sb[ci][:, lo:lo + MT], quart_col[:],
                                     start=(ci == 0), stop=(ci == 1))
                # per-column evac so tile gi's sqrt isn't gated on later tiles
                nc.vector.tensor_copy(fn2t[:, gi:gi + 1], pfnt[:, gi:gi + 1])

            b0 = slice(0, 3 * TT)
            t3 = slice(3 * TT, 4 * TT)
            # s = sqrt(d2' + |f|^2_col)  (ACT, per tile: bias is per-tile)
            for gi in range(NMT):
                bk = 0 if gi < 3 else 1
                w = slice(gi * TT, (gi + 1) * TT)
                pw = slice((gi - BO[bk]) * TT, (gi - BO[bk] + 1) * TT)
                nc.scalar.activation(s_all[:, w], pd2s[bk][:, pw], AF.Sqrt,
                                     bias=fn2t[:, gi:gi + 1])
            # exp(-s) bits: tiles 0-1 then tile 2 on DVE (pipelines behind
            # the per-tile sqrts), tile3 on ACT
            b01 = slice(0, 2 * TT)
            t2s = slice(2 * TT, 3 * TT)
            nc.vector.tensor_scalar(eb[:, b01], s_all[:, b01],
                                    -A_EXP, B_EXP, op0=AL.mult, op1=AL.add)
            e2 = eb[:, b01].bitcast(F32).rearrange("p (a b) -> p a b",
                                                   a=2, b=TT)
            nc.vector.tensor_reduce(z4[:, 0:2], e2, axis=AX.X, op=AL.add)
            nc.vector.tensor_scalar(eb[:, t2s], s_all[:, t2s],
                                    -A_EXP, B_EXP, op0=AL.mult, op1=AL.add)
            nc.vector.tensor_reduce(z4[:, 2:3], eb[:, t2s].bitcast(F32),
                                    axis=AX.X, op=AL.add)
            nc.scalar.activation(eb[:, t3], s_all[:, t3], AF.Identity,
                                 scale=-A_EXP, bias=bexp_col[:])
            nc.vector.tensor_reduce(z4[:, 3:4], eb[:, t3].bitcast(F32),
                                    axis=AX.X, op=AL.add)
            # prior - s on Pool (feeds the final per-tile bias adds)
            nc.gpsimd.tensor_sub(o_all[:, b0], prior_sb[:, b0], s_all[:, b0])
            nc.gpsimd.tensor_sub(o_all[:, t3], prior_sb[:, t3], s_all[:, t3])
            # -lnz = -LN_MUL * float(bits(z)) - LN_ADD (int32 input is
            # converted by the DVE datapath, so no separate cast op)
            nc.vector.tensor_scalar(lnzn4[:], z4[:].bitcast(I32),
                                    -LN_MUL, -LN_ADD,
                                    op0=AL.mult, op1=AL.add)
            # o += -lnz: tiles 0,1 on DVE, tiles 2,3 on ACT; halves leave
            # on the two hw queues (per-tile DMAs pay too much issue cost)
            for gi in range(NMT):
                w = slice(gi * TT, (gi + 1) * TT)
                if gi < 2:
                    nc.vector.tensor_scalar(o_all[:, w], o_all[:, w],
                                            lnzn4[:, gi:gi + 1], None,
                                            op0=AL.add)
                else:
                    nc.scalar.activation(o_all[:, w], o_all[:, w], AF.Identity,
                                         bias=lnzn4[:, gi:gi + 1])
                if gi == 1:
                    nc.sync.dma_start(d_out.ap()[:, 0:2 * TT],
                                      o_all[:, 0:2 * TT])
            nc.scalar.dma_start(d_out.ap()[:, 2 * TT:4 * TT],
                                o_all[:, 2 * TT:4 * TT])

    _split_multiwait(nc)
    _nc_cache = nc
    return nc


# ------------------------------------------------------------------ host glue
def _h2(a):
    """(256, X) -> (128, 2X): ci chunk c at columns [c*X, (c+1)*X)."""
    return np.concatenate([a[:128], a[128:]], axis=1)


def _prep_shared(t_w1, t_b1, t_w2, t_b2, f_w1, f_b1, f_w2, f_b2, f_w3, f_b3):
    tw1h = np.asarray(t_w1, np.float32).transpose(1, 2, 0).reshape(ADIM, 3 * ADIM)
    tw2h = np.asarray(t_w2, np.float32)[:, :, 0].T
    fw1h = np.asarray(f_w1, np.float32).transpose(1, 2, 0).reshape(ODIM, 3 * ADIM)
    fw2h = np.asarray(f_w2, np.float32).transpose(1, 2, 0).reshape(ADIM, 3 * ADIM)
    fw3h = np.asarray(f_w3, np.float32)[:, :, 0].T

    f2pack = np.ascontiguousarray(_h2(fw2h)).astype(np.float16)
    wdpack = np.empty((128, WD_W), np.float16)
    wdpack[:, WD_TW2:WD_TW2 + 2 * 256] = _h2(tw2h).astype(np.float16)
    wdpack[:, WD_FW3:WD_FW3 + 2 * 256] = _h2(fw3h).astype(np.float16)

    biases = np.zeros((256, 8), np.float32)
    for j, v in enumerate([t_b1, t_b2, f_b1, f_b2, f_b3,
                           -2.0 * np.asarray(f_b3)]):
        biases[:, j] = np.asarray(v, np.float32)

    return {
        "f2pack": f2pack,
        "wdpack": wdpack,
        "tw1h2": _h2(tw1h).astype(np.float16),
        "fw1h": fw1h.astype(np.float16),
        "biases2": _h2(biases),          # (128, 16)
    }


def _prep_core_inputs(c, text, feats, x_masks, shared):
    b, h = divmod(c, 2)
    s = h * HALF

    bm = np.zeros((128, BM_W), np.float32)
    bm[:, 0:16] = shared["biases2"]
    bm[:, BM_MASK2] = 0.0 if s - 1 < 0 else 1.0
    bm[:, BM_MASK2 + 1] = 0.0 if s + HALF >= T_FEATS else 1.0
    bm[0, BM_TMROW:BM_TMROW + TT] = MASK_PENALTY * x_masks[b].astype(np.float32)

    fpack = np.zeros((ODIM, FP_W), np.float16)
    lo, hi = max(0, s - 2), min(T_FEATS, s + TFW)
    fpack[:, lo - (s - 2):hi - (s - 2)] = feats[b, lo:hi].T.astype(np.float16)
    fpack[:, FP_FW1:] = shared["fw1h"]

    textT = np.zeros((ADIM, TTW), np.float16)
    textT[:, 1:1 + TT] = text[b].T.astype(np.float16)
    tpack = np.empty((128, TP_W), np.float16)
    tpack[:, TP_TEXT:TP_TEXT + 2 * TTW] = _h2(textT)
    tpack[:, TP_TW1:] = shared["tw1h2"]

    prior = _beta_binomial_prior()[s:s + HALF]               # (400, 160)
    prior_pack = np.ascontiguousarray(
        prior.reshape(NMT, MT, TT).transpose(1, 0, 2).reshape(MT, NMT * TT)
    )

    return {
        "biasmask": bm,
        "fpack": fpack,
        "tpack": tpack,
        "f2pack": shared["f2pack"],
        "wdpack": shared["wdpack"],
        "prior": prior_pack,
    }


def kernel(text, feats, text_lengths, feats_lengths, x_masks,
           t_w1, t_b1, t_w2, t_b2, f_w1, f_b1, f_w2, f_b2, f_w3, f_b3):
    text = np.asarray(text, np.float32)
    feats = np.asarray(feats, np.float32)
    x_masks = np.asarray(x_masks)

    shared = _prep_shared(t_w1, t_b1, t_w2, t_b2,
                          f_w1, f_b1, f_w2, f_b2, f_w3, f_b3)
    nc = _build_program()
    in_maps = [_prep_core_inputs(c, text, feats, x_masks, shared)
               for c in range(N_CORES)]
    res = None
    last_exc = None
    for _attempt in range(3):
        try:
            res = run_bass_kernel_spmd(nc, in_maps,
                                       core_ids=list(range(N_CORES)))
            break
        except Exception as e:   # transient NRT exec-unit flake on cold NEFFs
            last_exc = e
    if res is None:
        raise last_exc

    out = np.empty((B, T_FEATS, T_TEXT), np.float32)
    for c in range(N_CORES):
        b, h = divmod(c, 2)
        blk = res.results[c]["out"].reshape(MT, NMT, TT).transpose(1, 0, 2)
        out[b, h * HALF:(h + 1) * HALF, :] = blk.reshape(HALF, TT)
    return out
